# revision 1
# baseline (speedup 1.0000x reference)
"""Trainium2 Bass kernel for nn_NodeModel (GNN scatter-mean + node MLP).

Self-contained: takes FULL inputs as numpy arrays, shards by destination-node
range across 8 NeuronCores, runs a Bass/Tile kernel per core via
run_bass_kernel_spmd, and reassembles the full [500000, 8] output.

Strategy: nodes are sharded by destination range (62500 per core, no
collectives). The host sorts edges by destination and lays the per-edge
message [x[row] | edge_attr | 1] into a padded per-node slot stream
[node][17ch][G slots] in bf16. Each core dense-streams its slab, reduces over
the slot axis on the vector engine (f32 accumulation) to get per-node
sums+counts, then computes mean + the 2-layer MLP node-major on the vector
engine (u is folded into an effective bias on the host).

Layout: per-core padded node count NPAD = 128*NPP; node n -> partition n%128,
free column n//128.
"""
from contextlib import ExitStack

import numpy as np

import concourse.bacc as bacc
import concourse.mybir as mybir
import concourse.tile as tile
from concourse.bass_utils import run_bass_kernel_spmd

F_X = 8
F_E = 8
NCH = F_X + F_E + 1  # 17: x | attr | ones
H = 25
N_CORES = 8
N_NODES = 500000


def build_kernel(npp, G, chunk=16, mlp_split=3, repeat=1, stream_repeat=1):
    assert G % 8 == 0
    npad = 128 * npp
    nc = bacc.Bacc("TRN2", target_bir_lowering=False)

    streamP = nc.dram_tensor("streamP", [npad, NCH, G], mybir.dt.bfloat16,
                             kind="ExternalInput")
    xownT = nc.dram_tensor("xownT", [128, npp, F_X], mybir.dt.float32,
                           kind="ExternalInput")
    w1b = nc.dram_tensor("w1b", [128, H - 1, H], mybir.dt.float32,
                         kind="ExternalInput")
    b1b = nc.dram_tensor("b1b", [128, H], mybir.dt.float32, kind="ExternalInput")
    w2b = nc.dram_tensor("w2b", [128, H, F_X], mybir.dt.float32,
                         kind="ExternalInput")
    b2b = nc.dram_tensor("b2b", [128, F_X], mybir.dt.float32,
                         kind="ExternalInput")
    out = nc.dram_tensor("out", [128, npp, F_X], mybir.dt.float32,
                         kind="ExternalOutput")

    chunks = []
    s = 0
    while s < npp:
        c = min(chunk, npp - s)
        chunks.append((s, c))
        s += c

    with tile.TileContext(nc) as tc, ExitStack() as ctx:
        const = ctx.enter_context(tc.tile_pool(name="const", bufs=1))
        accp = ctx.enter_context(tc.tile_pool(name="accum", bufs=1))
        sp = ctx.enter_context(tc.tile_pool(name="stream", bufs=3))
        mlpp = ctx.enter_context(tc.tile_pool(name="mlp", bufs=1))

        accum = accp.tile([128, npp, NCH], mybir.dt.float32)

        w1t = const.tile([128, H - 1, H], mybir.dt.float32)
        nc.sync.dma_start(out=w1t[:], in_=w1b[:])
        b1t = const.tile([128, H], mybir.dt.float32)
        nc.sync.dma_start(out=b1t[:], in_=b1b[:])
        w2t = const.tile([128, H, F_X], mybir.dt.float32)
        nc.sync.dma_start(out=w2t[:], in_=w2b[:])
        b2t = const.tile([128, F_X], mybir.dt.float32)
        nc.sync.dma_start(out=b2t[:], in_=b2b[:])
        xo = const.tile([128, npp, F_X], mybir.dt.float32)
        nc.sync.dma_start(out=xo[:], in_=xownT[:])
        outt = const.tile([128, npp, F_X], mybir.dt.float32)
        inv = const.tile([128, npp], mybir.dt.float32)

        chunks = chunks * stream_repeat
        for _ in range(repeat):
            for (s0, cs) in chunks:
                st = sp.tile([128, chunk, NCH, G], mybir.dt.bfloat16, tag="st")
                nc.sync.dma_start(
                    out=st[:, :cs, :, :].rearrange("p c f g -> p c (f g)"),
                    in_=streamP.ap().rearrange("(s p) c g -> p s (c g)",
                                               p=128)[:, s0:s0 + cs, :],
                )
                nc.vector.reduce_sum(
                    out=accum[:, s0:s0 + cs, :],
                    in_=st[:, :cs, :, :],
                    axis=mybir.AxisListType.X,
                )

            # ---- mean + MLP (node-major on vector engine) ----
            nc.vector.tensor_scalar_max(out=inv[:], in0=accum[:, :, NCH - 1],
                                        scalar1=1.0)
            nc.vector.reciprocal(out=inv[:], in_=inv[:])

            msz = (npp + mlp_split - 1) // mlp_split
            for m0 in range(0, npp, msz):
                mc = min(msz, npp - m0)
                feat = mlpp.tile([128, msz, H - 1], mybir.dt.float32, tag="feat")
                nc.vector.tensor_copy(out=feat[:, :mc, 0:F_X],
                                      in_=xo[:, m0:m0 + mc, :])
                for ci in range(F_X + F_E):
                    nc.vector.tensor_tensor(
                        out=feat[:, :mc, F_X + ci],
                        in0=accum[:, m0:m0 + mc, ci],
                        in1=inv[:, m0:m0 + mc],
                        op=mybir.AluOpType.mult,
                    )
                h = mlpp.tile([128, msz, H], mybir.dt.float32, tag="h")
                for j in range(H):
                    nc.vector.scalar_tensor_tensor(
                        out=h[:, :mc, j],
                        in0=feat[:, :mc, 0],
                        scalar=w1t[:, 0, j:j + 1],
                        in1=b1t[:, j:j + 1].to_broadcast([128, mc]),
                        op0=mybir.AluOpType.mult,
                        op1=mybir.AluOpType.add,
                    )
                    for k in range(1, H - 1):
                        nc.vector.scalar_tensor_tensor(
                            out=h[:, :mc, j],
                            in0=feat[:, :mc, k],
                            scalar=w1t[:, k, j:j + 1],
                            in1=h[:, :mc, j],
                            op0=mybir.AluOpType.mult,
                            op1=mybir.AluOpType.add,
                        )
                nc.vector.tensor_scalar_max(out=h[:, :mc, :], in0=h[:, :mc, :],
                                            scalar1=0.0)
                for cch in range(F_X):
                    nc.vector.scalar_tensor_tensor(
                        out=outt[:, m0:m0 + mc, cch],
                        in0=h[:, :mc, 0],
                        scalar=w2t[:, 0, cch:cch + 1],
                        in1=b2t[:, cch:cch + 1].to_broadcast([128, mc]),
                        op0=mybir.AluOpType.mult,
                        op1=mybir.AluOpType.add,
                    )
                    for j in range(1, H):
                        nc.vector.scalar_tensor_tensor(
                            out=outt[:, m0:m0 + mc, cch],
                            in0=h[:, :mc, j],
                            scalar=w2t[:, j, cch:cch + 1],
                            in1=outt[:, m0:m0 + mc, cch],
                            op0=mybir.AluOpType.mult,
                            op1=mybir.AluOpType.add,
                        )
        nc.sync.dma_start(out=out[:], in_=outt[:])

    nc.compile()
    return nc


def _to_bf16_bytes(a_f32):
    """f32 -> bf16 (round-to-nearest-even) as uint16 view."""
    u = a_f32.view(np.uint32)
    rounded = (u + 0x7FFF + ((u >> 16) & 1)) >> 16
    return rounded.astype(np.uint16)


def prep_core_inputs(x, row, col, edge_attr, W1, b1, W2, b2, u,
                     n_nodes=N_NODES, n_cores=N_CORES, G=None):
    n_per = n_nodes // n_cores
    deg = np.bincount(col, minlength=n_nodes)
    maxdeg = int(deg.max()) if len(col) else 0
    if G is None:
        G = max(8, (maxdeg + 7) // 8 * 8)
    assert maxdeg <= G, (maxdeg, G)

    order = np.argsort(col, kind="stable")
    sc = col[order]
    within = np.arange(len(col), dtype=np.int64) - \
        np.concatenate([[0], np.cumsum(deg)[:-1]])[sc]

    # message = [x[row] | attr | 1] per edge, bf16
    msg = np.empty((len(col), NCH), np.float32)
    msg[:, :F_X] = x[row[order]]
    msg[:, F_X:F_X + F_E] = edge_attr[order]
    msg[:, NCH - 1] = 1.0
    msg16 = _to_bf16_bytes(msg)

    stream = np.zeros((n_nodes, NCH, G), np.uint16)
    stream[sc, :, within] = msg16

    b1_eff = (b1 + u[0] * W1[H - 1]).astype(np.float32)
    w1b = np.ascontiguousarray(np.broadcast_to(W1[:H - 1], (128, H - 1, H)),
                               np.float32)
    b1b = np.ascontiguousarray(np.broadcast_to(b1_eff, (128, H)), np.float32)
    w2b = np.ascontiguousarray(np.broadcast_to(W2, (128, H, F_X)), np.float32)
    b2b = np.ascontiguousarray(np.broadcast_to(b2, (128, F_X)), np.float32)

    npp = (n_per + 127) // 128
    npad = npp * 128

    in_maps = []
    for c in range(n_cores):
        lo = c * n_per
        hi = lo + n_per
        s_c = np.zeros((npad, NCH, G), np.uint16)
        s_c[:n_per] = stream[lo:hi]
        xo_c = np.zeros((npad, F_X), np.float32)
        xo_c[:n_per] = x[lo:hi]
        in_maps.append({
            "streamP": s_c.view(np.dtype("uint16")),
            "xownT": np.ascontiguousarray(
                xo_c.reshape(npp, 128, F_X).transpose(1, 0, 2)),
            "w1b": w1b, "b1b": b1b, "w2b": w2b, "b2b": b2b,
        })
    meta = dict(G=G, npp=npp, npad=npad, n_per=n_per)
    return in_maps, meta


def assemble_output(results, meta, n_nodes=N_NODES, n_cores=N_CORES):
    n_per = n_nodes // n_cores
    parts = []
    for c in range(n_cores):
        o = results[c]["out"]
        o = o.transpose(1, 0, 2).reshape(meta["npad"], F_X)[:n_per]
        parts.append(o)
    return np.concatenate(parts, 0)


LAST_RUN = {}


def kernel(x, edge_index, edge_attr, u, batch, W1, b1, W2, b2):
    x = np.asarray(x, np.float32)
    edge_attr = np.asarray(edge_attr, np.float32)
    u = np.asarray(u, np.float32)
    W1 = np.asarray(W1, np.float32)
    b1 = np.asarray(b1, np.float32)
    W2 = np.asarray(W2, np.float32)
    b2 = np.asarray(b2, np.float32)
    row = np.asarray(edge_index[0]).astype(np.int32)
    col = np.asarray(edge_index[1]).astype(np.int32)

    in_maps, meta = prep_core_inputs(x, row, col, edge_attr, W1, b1, W2, b2, u)
    nc = build_kernel(meta["npp"], meta["G"])
    # bf16 tensors are passed as uint16; bass expects ml_dtypes bfloat16 view
    import ml_dtypes
    for m in in_maps:
        m["streamP"] = m["streamP"].view(ml_dtypes.bfloat16)
    res = run_bass_kernel_spmd(nc, in_maps, core_ids=list(range(N_CORES)))
    LAST_RUN.update(nc=nc, in_maps=in_maps, meta=meta)
    return assemble_output(res.results, meta).astype(np.float32)



# revision 4
# speedup vs baseline: 1.1757x; 1.1757x over previous
"""Trainium2 Bass kernel for nn_NodeModel (GNN scatter-mean + node MLP).

Self-contained: takes FULL inputs as numpy arrays, shards by destination-node
range across 8 NeuronCores, runs a Bass/Tile kernel per core via
run_bass_kernel_spmd, and reassembles the full [500000, 8] output.

Strategy: nodes sharded by destination range (62500/core, no collectives).
The host sorts edges by destination, degree-sorts nodes within each core, and
packs the per-edge message [x[row] | edge_attr] (16 ch, bf16) into per-chunk
slot arrays whose slot count G tracks the local max degree (~33 avg instead of
the global max ~70), laid out partition-major so every stream DMA is
[128 partitions x large-contiguous].  Per-node counts (already computed for
the layout) ship as a tiny side input.

Device per core: chunked DMA -> one reduce_sum per chunk (DVE) over the slot
axis -> mean via max/reciprocal/multiply -> PE transposes of 128-node feature
columns -> PE matmuls for the 2-layer MLP (W1 24x25, W2 25x8, bf16), ACT for
bias+ReLU and PSUM evacuation.  Output is [8, npad] channel-major; the host
transposes and un-permutes the degree sort.
"""
from contextlib import ExitStack

import numpy as np

import concourse.bacc as bacc
import concourse.mybir as mybir
import concourse.tile as tile
from concourse.bass_utils import run_bass_kernel_spmd
from concourse.masks import make_identity

F_X = 8
F_E = 8
NCH = F_X + F_E          # 16 summed message channels
HF = F_X + NCH           # 24 feature channels into the MLP
H = 25
N_CORES = 8
N_NODES = 500_000
N_PER = N_NODES // N_CORES   # 62500
NQ = 4                       # quarters (pipeline granularity)
NPP = 492                    # node columns per core (492*128 = 62976 >= 62500)
L_BUDGET = 8448              # bf16 elems per partition per stream chunk


def plan_chunks(env, npp, nq, l_budget=L_BUDGET):
    """env: [npp*128] descending max-degree envelope (shared across cores).
    Returns ([(q, col_in_q, C, G, off)], total_W). One chunk = C node columns
    sharing slot count G; per-partition layout [ch][col][slot]."""
    qc = npp // nq
    chunks = []
    off = 0
    for q in range(nq):
        col = 0
        while col < qc:
            g = max(1, int(env[(q * qc + col) * 128]))
            c = max(1, min(qc - col, l_budget // (NCH * g)))
            chunks.append((q, col, c, g, off))
            off += NCH * c * g
            col += c
    return chunks, off


def build_kernel(npp, nq, chunks, W, repeat=1):
    qc = npp // nq
    dt = mybir.dt
    nc = bacc.Bacc("TRN2", target_bir_lowering=False)

    streamP = nc.dram_tensor("streamP", [128, W], dt.bfloat16,
                             kind="ExternalInput")
    xq = nc.dram_tensor("xq", [128, nq, F_X, qc], dt.float32,
                        kind="ExternalInput")
    cntq = nc.dram_tensor("cntq", [128, nq, qc], dt.float32,
                          kind="ExternalInput")
    w1 = nc.dram_tensor("w1", [HF, H], dt.bfloat16, kind="ExternalInput")
    b1 = nc.dram_tensor("b1", [H, 1], dt.float32, kind="ExternalInput")
    w2 = nc.dram_tensor("w2", [H, F_X], dt.bfloat16, kind="ExternalInput")
    b2 = nc.dram_tensor("b2", [F_X, 1], dt.float32, kind="ExternalInput")
    outP = nc.dram_tensor("outP", [F_X, npp * 128], dt.float32,
                          kind="ExternalOutput")

    st_size = max(L_BUDGET, max(NCH * c * g for (_, _, c, g, _) in chunks))
    relu = mybir.ActivationFunctionType.Relu
    identf = mybir.ActivationFunctionType.Identity

    with tile.TileContext(nc) as tc, ExitStack() as ctx:
        const = ctx.enter_context(tc.tile_pool(name="const", bufs=1))
        persist = ctx.enter_context(tc.tile_pool(name="persist", bufs=1))
        sp = ctx.enter_context(tc.tile_pool(name="stream", bufs=3))
        msb = ctx.enter_context(tc.tile_pool(name="mlp", bufs=2))
        obp = ctx.enter_context(tc.tile_pool(name="outb", bufs=2))
        psum = ctx.enter_context(tc.tile_pool(name="psum", bufs=2,
                                              space="PSUM"))

        ident = const.tile([128, 128], dt.float32)
        make_identity(nc, ident)
        w1t = const.tile([HF, H], dt.bfloat16)
        nc.sync.dma_start(out=w1t[:], in_=w1[:])
        b1t = const.tile([H, 1], dt.float32)
        nc.sync.dma_start(out=b1t[:], in_=b1[:])
        w2t = const.tile([H, F_X], dt.bfloat16)
        nc.sync.dma_start(out=w2t[:], in_=w2[:])
        b2t = const.tile([F_X, 1], dt.float32)
        nc.sync.dma_start(out=b2t[:], in_=b2[:])

        by_q = {q: [ch for ch in chunks if ch[0] == q] for q in range(nq)}

        for q in [q for _ in range(repeat) for q in range(nq)]:
            feat = persist.tile([128, HF, qc], dt.float32, tag=f"feat{q}")
            accum = persist.tile([128, NCH, qc], dt.float32, tag=f"acc{q}")
            inv = persist.tile([128, qc], dt.float32, tag=f"inv{q}")

            nc.sync.dma_start(out=feat[:, 0:F_X, :], in_=xq[:, q])
            nc.sync.dma_start(out=inv[:], in_=cntq[:, q])
            nc.vector.tensor_scalar_max(out=inv[:], in0=inv[:], scalar1=1.0)
            nc.vector.reciprocal(out=inv[:], in_=inv[:])

            for (_, col, c, g, off) in by_q[q]:
                stt = sp.tile([128, st_size], dt.bfloat16, tag="st")
                n = NCH * c * g
                nc.sync.dma_start(out=stt[:, :n], in_=streamP[:, off:off + n])
                nc.vector.reduce_sum(
                    out=accum[:, :, col:col + c],
                    in_=stt[:, :n].rearrange("p (f c g) -> p f c g",
                                             f=NCH, c=c),
                    axis=mybir.AxisListType.X,
                )

            for ci in range(NCH):
                nc.vector.tensor_tensor(
                    out=feat[:, F_X + ci, :], in0=accum[:, ci, :], in1=inv[:],
                    op=mybir.AluOpType.mult,
                )

            # ---- MLP over this quarter, blocks of up to 4 node columns ----
            ob = None
            ob_base = 0
            for b0 in range(0, qc, 4):
                bc = min(4, qc - b0)
                n = bc * 128
                if (b0 // 4) % 4 == 0:
                    ob = obp.tile([F_X, 2048], dt.float32, tag="ob")
                    ob_base = b0
                ftp = psum.tile([HF, 512], dt.float32, tag="ft")
                for i in range(bc):
                    nc.tensor.transpose(ftp[:, i * 128:(i + 1) * 128],
                                        feat[:, :, b0 + i], ident)
                fts = msb.tile([HF, 512], dt.bfloat16, tag="fts")
                nc.scalar.copy(out=fts[:, :n], in_=ftp[:, :n])
                hp = psum.tile([H, 512], dt.float32, tag="h")
                nc.tensor.matmul(hp[:, :n], w1t[:], fts[:, :n],
                                 start=True, stop=True)
                hs = msb.tile([H, 512], dt.bfloat16, tag="hs")
                nc.scalar.activation(hs[:, :n], hp[:, :n], relu, bias=b1t[:])
                op_ = psum.tile([F_X, 512], dt.float32, tag="o")
                nc.tensor.matmul(op_[:, :n], w2t[:], hs[:, :n],
                                 start=True, stop=True)
                oc = (b0 - ob_base) * 128
                nc.scalar.activation(ob[:, oc:oc + n], op_[:, :n], identf,
                                     bias=b2t[:])
                if (b0 // 4) % 4 == 3 or b0 + bc >= qc:
                    done = (b0 + bc - ob_base) * 128
                    base = (q * qc + ob_base) * 128
                    nc.sync.dma_start(out=outP[:, base:base + done],
                                      in_=ob[:, :done])

    nc.compile()
    return nc


def _to_bf16(a_f32):
    """f32 -> bf16 (round-to-nearest-even) as uint16 view."""
    u = np.ascontiguousarray(a_f32).view(np.uint32)
    rounded = (u + 0x7FFF + ((u >> 16) & 1)) >> 16
    return rounded.astype(np.uint16)


def prep_core_inputs(x, row, col, edge_attr, W1, b1, W2, b2, u,
                     n_nodes=N_NODES, n_cores=N_CORES, npp=NPP, nq=NQ):
    n_per = n_nodes // n_cores
    npad = npp * 128
    qc = npp // nq
    deg = np.bincount(col, minlength=n_nodes).astype(np.int64)

    # per-core degree sort; shared descending max-degree envelope
    orders = []
    dsort = np.zeros((n_cores, npad), np.int64)
    for c in range(n_cores):
        d = deg[c * n_per:(c + 1) * n_per]
        o = np.argsort(-d, kind="stable")
        orders.append(o)
        dsort[c, :n_per] = d[o]
    env = dsort.max(axis=0)
    chunks, W = plan_chunks(env, npp, nq)

    # per-column lookup tables for the slot layout
    col2off = np.zeros(npp, np.int64)
    col2g = np.zeros(npp, np.int64)
    col2cg = np.zeros(npp, np.int64)
    col2cola = np.zeros(npp, np.int64)
    for (q, colq, c, g, off) in chunks:
        c0 = q * qc + colq
        for k in range(c):
            col2off[c0 + k] = off
            col2g[c0 + k] = g
            col2cg[c0 + k] = c * g
            col2cola[c0 + k] = k

    # edges sorted by destination
    order = np.argsort(col, kind="stable")
    sc = col[order]
    starts = np.zeros(n_nodes + 1, np.int64)
    starts[1:] = np.cumsum(deg)
    within = np.arange(len(col), dtype=np.int64) - starts[sc]
    msg = np.empty((len(col), NCH), np.float32)
    msg[:, :F_X] = x[row[order]]
    msg[:, F_X:] = edge_attr[order]
    msg16 = _to_bf16(msg)

    b1_eff = (b1 + u[0] * W1[HF]).astype(np.float32).reshape(H, 1)
    w1_16 = _to_bf16(np.ascontiguousarray(W1[:HF].astype(np.float32)))
    w2_16 = _to_bf16(np.ascontiguousarray(W2.astype(np.float32)))
    b2_c = np.ascontiguousarray(b2.astype(np.float32).reshape(F_X, 1))

    bounds = np.searchsorted(sc, np.arange(0, n_nodes + 1, n_per))
    in_maps = []
    for c in range(n_cores):
        o = orders[c]
        rank = np.empty(n_per, np.int64)
        rank[o] = np.arange(n_per)
        e0, e1 = bounds[c], bounds[c + 1]
        r = rank[sc[e0:e1] - c * n_per]
        p = r & 127
        colg = r >> 7
        pos0 = col2off[colg] + col2cola[colg] * col2g[colg] + within[e0:e1]
        cg = col2cg[colg]
        stream = np.zeros((128, W), np.uint16)
        for ch in range(NCH):
            stream[p, pos0 + ch * cg] = msg16[e0:e1, ch]

        xs = np.zeros((npad, F_X), np.float32)
        xs[:n_per] = x[c * n_per:(c + 1) * n_per][o]
        cnts = np.zeros(npad, np.float32)
        cnts[:n_per] = deg[c * n_per:(c + 1) * n_per][o]
        # rank r -> partition r%128, column r//128; [128, nq, F_X, qc]
        xq_arr = xs.reshape(nq, qc, 128, F_X).transpose(2, 0, 3, 1)
        cq_arr = cnts.reshape(nq, qc, 128).transpose(2, 0, 1)
        in_maps.append({
            "streamP": stream,
            "xq": np.ascontiguousarray(xq_arr),
            "cntq": np.ascontiguousarray(cq_arr),
            "w1": w1_16, "b1": b1_eff, "w2": w2_16, "b2": b2_c,
        })
    meta = dict(chunks=chunks, W=W, orders=orders, npp=npp, nq=nq)
    return in_maps, meta


def assemble_output(results, meta, n_nodes=N_NODES, n_cores=N_CORES):
    n_per = n_nodes // n_cores
    parts = []
    for c in range(n_cores):
        o = results[c]["outP"]  # [F_X, npad]
        res = np.empty((n_per, F_X), np.float32)
        res[meta["orders"][c]] = o[:, :n_per].T
        parts.append(res)
    return np.concatenate(parts, 0)


LAST_RUN = {}


def kernel(x, edge_index, edge_attr, u, batch, W1, b1, W2, b2):
    x = np.asarray(x, np.float32)
    edge_attr = np.asarray(edge_attr, np.float32)
    u = np.asarray(u, np.float32)
    W1 = np.asarray(W1, np.float32)
    b1 = np.asarray(b1, np.float32)
    W2 = np.asarray(W2, np.float32)
    b2 = np.asarray(b2, np.float32)
    row = np.asarray(edge_index[0]).astype(np.int64)
    col = np.asarray(edge_index[1]).astype(np.int64)

    in_maps, meta = prep_core_inputs(x, row, col, edge_attr, W1, b1, W2, b2, u)
    nc = build_kernel(meta["npp"], meta["nq"], meta["chunks"], meta["W"])
    import ml_dtypes
    for m in in_maps:
        m["streamP"] = m["streamP"].view(ml_dtypes.bfloat16)
        m["w1"] = m["w1"].view(ml_dtypes.bfloat16)
        m["w2"] = m["w2"].view(ml_dtypes.bfloat16)
    res = run_bass_kernel_spmd(nc, in_maps, core_ids=list(range(N_CORES)))
    LAST_RUN.update(nc=nc, in_maps=in_maps, meta=meta)
    return assemble_output(res.results, meta).astype(np.float32)


# revision 21
# speedup vs baseline: 1.2080x; 1.0275x over previous
"""Trainium2 Bass kernel for nn_NodeModel (GNN scatter-mean + node MLP).

Self-contained: takes FULL inputs as numpy arrays, shards by destination-node
range across 8 NeuronCores, runs a Bass/Tile kernel per core via
run_bass_kernel_spmd, and reassembles the full [500000, 8] output.

Strategy: nodes sharded by destination range (62500/core, no collectives).
The host sorts edges by destination, degree-sorts nodes within each core, and
packs the per-edge message [x[row] | edge_attr] (16 ch, bf16) into per-chunk
slot arrays whose slot count G tracks the local max degree (~33 avg instead of
the global max ~70), laid out partition-major so every stream DMA is
[128 partitions x large-contiguous].  Per-node counts (already computed for
the layout) ship as a tiny side input.

Device per core: chunked DMA -> one reduce_sum per chunk (DVE) over the slot
axis -> mean via max/reciprocal/multiply -> PE transposes of 128-node feature
columns -> PE matmuls for the 2-layer MLP (W1 24x25, W2 25x8, bf16), ACT for
bias+ReLU and PSUM evacuation.  Output is [8, npad] channel-major; the host
transposes and un-permutes the degree sort.
"""
from contextlib import ExitStack

import numpy as np

import concourse.bacc as bacc
import concourse.mybir as mybir
import concourse.tile as tile
from concourse.bass_utils import run_bass_kernel_spmd
from concourse.masks import make_identity

F_X = 8
F_E = 8
NCH = F_X + F_E          # 16 summed message channels
HF = F_X + NCH           # 24 feature channels into the MLP
H = 25
N_CORES = 8
N_NODES = 500_000
N_PER = N_NODES // N_CORES   # 62500
NQ = 4                       # quarters (pipeline granularity)
NPP = 492                    # node columns per core (492*128 = 62976 >= 62500)
L_BUDGET = 8448              # bf16 elems per partition per stream chunk


def plan_chunks(env, npp, nq, l_budget=L_BUDGET):
    """env: [npp*128] descending max-degree envelope (shared across cores).
    Returns ([(q, col_in_q, C, G, off)], total_W). One chunk = C node columns
    sharing slot count G; per-partition layout [ch][col][slot]."""
    qc = npp // nq
    chunks = []
    off = 0
    for q in range(nq):
        col = 0
        while col < qc:
            g = max(1, int(env[(q * qc + col) * 128]))
            c = max(1, min(qc - col, l_budget // (NCH * g)))
            chunks.append((q, col, c, g, off))
            off += NCH * c * g
            col += c
    return chunks, off


def build_kernel(npp, nq, chunks, W, repeat=1, do_reduce=True, do_mlp=True):
    qc = npp // nq
    dt = mybir.dt
    nc = bacc.Bacc("TRN2", target_bir_lowering=False)

    streamP = nc.dram_tensor("streamP", [128, W], dt.bfloat16,
                             kind="ExternalInput")
    xq = nc.dram_tensor("xq", [128, nq, F_X, qc], dt.float32,
                        kind="ExternalInput")
    cntq = nc.dram_tensor("cntq", [128, nq, qc], dt.float32,
                          kind="ExternalInput")
    w1 = nc.dram_tensor("w1", [HF, H], dt.bfloat16, kind="ExternalInput")
    b1 = nc.dram_tensor("b1", [H, 1], dt.float32, kind="ExternalInput")
    w2 = nc.dram_tensor("w2", [H, F_X], dt.bfloat16, kind="ExternalInput")
    b2 = nc.dram_tensor("b2", [F_X, 1], dt.float32, kind="ExternalInput")
    outP = nc.dram_tensor("outP", [F_X, npp * 128], dt.float32,
                          kind="ExternalOutput")

    st_size = max(L_BUDGET, max(NCH * c * g for (_, _, c, g, _) in chunks))
    relu = mybir.ActivationFunctionType.Relu
    identf = mybir.ActivationFunctionType.Identity

    with tile.TileContext(nc) as tc, ExitStack() as ctx:
        const = ctx.enter_context(tc.tile_pool(name="const", bufs=1))
        persist = ctx.enter_context(tc.tile_pool(name="persist", bufs=1))
        sp = ctx.enter_context(tc.tile_pool(name="stream", bufs=3))
        msb = ctx.enter_context(tc.tile_pool(name="mlp", bufs=2))
        obp = ctx.enter_context(tc.tile_pool(name="outb", bufs=2))
        psum = ctx.enter_context(tc.tile_pool(name="psum", bufs=2,
                                              space="PSUM"))

        ident = const.tile([128, 128], dt.float32)
        make_identity(nc, ident)
        w1t = const.tile([HF, H], dt.bfloat16)
        nc.sync.dma_start(out=w1t[:], in_=w1[:])
        b1t = const.tile([H, 1], dt.float32)
        nc.sync.dma_start(out=b1t[:], in_=b1[:])
        w2t = const.tile([H, F_X], dt.bfloat16)
        nc.sync.dma_start(out=w2t[:], in_=w2[:])
        b2t = const.tile([F_X, 1], dt.float32)
        nc.sync.dma_start(out=b2t[:], in_=b2[:])

        by_q = {q: [ch for ch in chunks if ch[0] == q] for q in range(nq)}

        for q in [q for _ in range(repeat) for q in range(nq)]:
            feat = persist.tile([128, HF, qc], dt.float32, tag=f"feat{q}")
            accum = persist.tile([128, NCH, qc], dt.float32, tag=f"acc{q}")
            inv = persist.tile([128, qc], dt.float32, tag=f"inv{q}")

            # scalar (ACT) HWDGE ring: keeps these off the SP ring so a
            # queued wait can't stall the stream DMAs behind it
            nc.scalar.dma_start(out=feat[:, 0:F_X, :], in_=xq[:, q])
            nc.scalar.dma_start(out=inv[:], in_=cntq[:, q])
            nc.vector.tensor_scalar_max(out=inv[:], in0=inv[:], scalar1=1.0)
            nc.vector.reciprocal(out=inv[:], in_=inv[:])

            if do_reduce:
                for (_, col, c, g, off) in by_q[q]:
                    stt = sp.tile([128, st_size], dt.bfloat16, tag="st")
                    n = NCH * c * g
                    nc.sync.dma_start(out=stt[:, :n],
                                      in_=streamP[:, off:off + n])
                    nc.vector.reduce_sum(
                        out=accum[:, :, col:col + c],
                        in_=stt[:, :n].rearrange("p (f c g) -> p f c g",
                                                 f=NCH, c=c),
                        axis=mybir.AxisListType.X,
                    )

                for ci in range(NCH):
                    nc.vector.tensor_tensor(
                        out=feat[:, F_X + ci, :], in0=accum[:, ci, :],
                        in1=inv[:], op=mybir.AluOpType.mult,
                    )

            if not do_mlp:  # timing probe only: skip MLP, output stays zero
                continue

            # ---- MLP over this quarter, blocks of up to 4 node columns ----
            ob = None
            ob_base = 0
            for b0 in range(0, qc, 4):
                bc = min(4, qc - b0)
                n = bc * 128
                if (b0 // 4) % 4 == 0:
                    ob = obp.tile([F_X, 2048], dt.float32, tag="ob")
                    ob_base = b0
                ftp = psum.tile([HF, 512], dt.float32, tag="ft")
                for i in range(bc):
                    nc.tensor.transpose(ftp[:, i * 128:(i + 1) * 128],
                                        feat[:, :, b0 + i], ident)
                fts = msb.tile([HF, 512], dt.bfloat16, tag="fts")
                nc.scalar.copy(out=fts[:, :n], in_=ftp[:, :n])
                hp = psum.tile([H, 512], dt.float32, tag="h")
                nc.tensor.matmul(hp[:, :n], w1t[:], fts[:, :n],
                                 start=True, stop=True)
                hs = msb.tile([H, 512], dt.bfloat16, tag="hs")
                nc.scalar.activation(hs[:, :n], hp[:, :n], relu, bias=b1t[:])
                op_ = psum.tile([F_X, 512], dt.float32, tag="o")
                nc.tensor.matmul(op_[:, :n], w2t[:], hs[:, :n],
                                 start=True, stop=True)
                oc = (b0 - ob_base) * 128
                nc.scalar.activation(ob[:, oc:oc + n], op_[:, :n], identf,
                                     bias=b2t[:])
                if (b0 // 4) % 4 == 3 or b0 + bc >= qc:
                    done = (b0 + bc - ob_base) * 128
                    base = (q * qc + ob_base) * 128
                    nc.scalar.dma_start(out=outP[:, base:base + done],
                                        in_=ob[:, :done])

    nc.compile()
    return nc


def _to_bf16(a_f32):
    """f32 -> bf16 (round-to-nearest-even) as uint16 view."""
    u = np.ascontiguousarray(a_f32).view(np.uint32)
    rounded = (u + 0x7FFF + ((u >> 16) & 1)) >> 16
    return rounded.astype(np.uint16)


def prep_core_inputs(x, row, col, edge_attr, W1, b1, W2, b2, u,
                     n_nodes=N_NODES, n_cores=N_CORES, npp=NPP, nq=NQ):
    n_per = n_nodes // n_cores
    npad = npp * 128
    qc = npp // nq
    deg = np.bincount(col, minlength=n_nodes).astype(np.int64)

    # per-core degree sort; shared descending max-degree envelope
    orders = []
    dsort = np.zeros((n_cores, npad), np.int64)
    for c in range(n_cores):
        d = deg[c * n_per:(c + 1) * n_per]
        o = np.argsort(-d, kind="stable")
        orders.append(o)
        dsort[c, :n_per] = d[o]
    env = dsort.max(axis=0)
    chunks, W = plan_chunks(env, npp, nq)

    # per-column lookup tables for the slot layout
    col2off = np.zeros(npp, np.int64)
    col2g = np.zeros(npp, np.int64)
    col2cg = np.zeros(npp, np.int64)   # per-channel stride C*G
    col2cola = np.zeros(npp, np.int64)
    for (q, colq, c, g, off) in chunks:
        c0 = q * qc + colq
        for k in range(c):
            col2off[c0 + k] = off
            col2g[c0 + k] = g
            col2cg[c0 + k] = c * g
            col2cola[c0 + k] = k

    # edges sorted by destination (int32 keys; bf16-convert before gathering
    # to avoid GB-scale f32 temporaries)
    order = np.argsort(col.astype(np.int32), kind="stable")
    sc = col.astype(np.int32)[order]
    starts = np.zeros(n_nodes + 1, np.int64)
    starts[1:] = np.cumsum(deg)
    within = np.arange(len(col), dtype=np.int64) - starts[sc]
    x16 = _to_bf16(x.astype(np.float32))
    ea16 = _to_bf16(edge_attr.astype(np.float32))
    msg16 = np.empty((len(col), NCH), np.uint16)
    msg16[:, :F_X] = x16[row[order]]
    msg16[:, F_X:] = ea16[order]

    b1_eff = (b1 + u[0] * W1[HF]).astype(np.float32).reshape(H, 1)
    w1_16 = _to_bf16(np.ascontiguousarray(W1[:HF].astype(np.float32)))
    w2_16 = _to_bf16(np.ascontiguousarray(W2.astype(np.float32)))
    b2_c = np.ascontiguousarray(b2.astype(np.float32).reshape(F_X, 1))

    bounds = np.searchsorted(sc, np.arange(0, n_nodes + 1, n_per))
    in_maps = []
    for c in range(n_cores):
        o = orders[c]
        rank = np.empty(n_per, np.int64)
        rank[o] = np.arange(n_per)
        e0, e1 = bounds[c], bounds[c + 1]
        r = rank[sc[e0:e1].astype(np.int64) - c * n_per]
        p = r & 127
        colg = r >> 7
        pos0 = (col2off[colg] + col2cola[colg] * col2g[colg]
                + within[e0:e1])
        cg = col2cg[colg]
        stream = np.zeros((128, W), np.uint16)
        flat = (p * W + pos0)[:, None] + cg[:, None] * np.arange(NCH)
        stream.ravel()[flat] = msg16[e0:e1]

        xs = np.zeros((npad, F_X), np.float32)
        xs[:n_per] = x[c * n_per:(c + 1) * n_per][o]
        cnts = np.zeros(npad, np.float32)
        cnts[:n_per] = deg[c * n_per:(c + 1) * n_per][o]
        # rank r -> partition r%128, column r//128; [128, nq, F_X, qc]
        xq_arr = xs.reshape(nq, qc, 128, F_X).transpose(2, 0, 3, 1)
        cq_arr = cnts.reshape(nq, qc, 128).transpose(2, 0, 1)
        in_maps.append({
            "streamP": stream,
            "xq": np.ascontiguousarray(xq_arr),
            "cntq": np.ascontiguousarray(cq_arr),
            "w1": w1_16, "b1": b1_eff, "w2": w2_16, "b2": b2_c,
        })
    meta = dict(chunks=chunks, W=W, orders=orders, npp=npp, nq=nq)
    return in_maps, meta


def assemble_output(results, meta, n_nodes=N_NODES, n_cores=N_CORES):
    n_per = n_nodes // n_cores
    parts = []
    for c in range(n_cores):
        o = results[c]["outP"]  # [F_X, npad]
        res = np.empty((n_per, F_X), np.float32)
        res[meta["orders"][c]] = o[:, :n_per].T
        parts.append(res)
    return np.concatenate(parts, 0)


LAST_RUN = {}


def kernel(x, edge_index, edge_attr, u, batch, W1, b1, W2, b2):
    x = np.asarray(x, np.float32)
    edge_attr = np.asarray(edge_attr, np.float32)
    u = np.asarray(u, np.float32)
    W1 = np.asarray(W1, np.float32)
    b1 = np.asarray(b1, np.float32)
    W2 = np.asarray(W2, np.float32)
    b2 = np.asarray(b2, np.float32)
    row = np.asarray(edge_index[0]).astype(np.int64)
    col = np.asarray(edge_index[1]).astype(np.int64)

    in_maps, meta = prep_core_inputs(x, row, col, edge_attr, W1, b1, W2, b2, u)
    nc = build_kernel(meta["npp"], meta["nq"], meta["chunks"], meta["W"])
    import ml_dtypes
    for m in in_maps:
        m["streamP"] = m["streamP"].view(ml_dtypes.bfloat16)
        m["w1"] = m["w1"].view(ml_dtypes.bfloat16)
        m["w2"] = m["w2"].view(ml_dtypes.bfloat16)
    res = run_bass_kernel_spmd(nc, in_maps, core_ids=list(range(N_CORES)))
    LAST_RUN.update(nc=nc, in_maps=in_maps, meta=meta)
    return assemble_output(res.results, meta).astype(np.float32)
